# revision 14
# baseline (speedup 1.0000x reference)
"""CaiT (nn_Cait_78984448573778) forward on 8 trn2 NeuronCores.

Data-parallel over batch: each core runs the full model on 2 images.
Activations transposed in SBUF ([C on partitions, tokens free]).
Big GEMMs in fp8e4 DoubleRow (2 k-chunks per pass); weights pre-scaled x32
on host (recovered in the PSUM->SBUF op). LN scale/bias folded into the
following matmul weights on host. LN stats via float32r matmuls (1 cyc/col).
Talking-heads: pre-softmax mix folded into per-head-scaled q (bf16 scores);
post-softmax mix as fp8 DoubleRow matmuls against host-built scaled-identity
j-pairs, which also transpose the maps for the AV matmul. rstd computed as
exp(-0.5*ln(var+eps)) so Act stays on the ln/exp table (only Gelu switches).
PSUM->SBUF moves spread across Pool/DVE/Act to balance engines.
"""

from contextlib import ExitStack

import numpy as np
import ml_dtypes

import concourse.bass as bass
import concourse.mybir as mybir
import concourse.tile as tile
from concourse import bacc
from concourse.bass_utils import run_bass_kernel_spmd

F32 = mybir.dt.float32
F32R = mybir.dt.float32r
BF16 = mybir.dt.bfloat16
FP8 = mybir.dt.float8e4
AF = mybir.ActivationFunctionType
ALU = mybir.AluOpType
DR = mybir.MatmulPerfMode.DoubleRow
AXX = mybir.AxisListType.X

# model dims
B, C, DEPTH, HEADS, D2, NCLS = 16, 768, 12, 12, 2, 1000
P_, IMG = 16, 224
GH = IMG // P_          # 14
NP = GH * GH            # 196
HD = C // HEADS         # 64
SCALE = HD ** -0.5
EPS = 1e-6

NCORES = 8
BL = B // NCORES        # 2 images per core
TC = BL * NP            # 392 token-columns, col = b*196 + n
CH = C // 128           # 6 c-chunks
KP = CH // 2            # 3 DoubleRow k-pairs
FF = 4 * C              # 3072
FCH = FF // 128         # 24
BIGW = CH * FF          # 18432 cols of the big weight buffers
TN = NP + 1             # 197 tokens with cls
TCA = BL * TN           # 394 cols, col = b*197 + t (t=0 is cls)
NCH = [(0, 128), (128, 68)]    # n-chunks of 196
TCH = [(0, 128), (128, 69)]    # tok-chunks of 197
TCP = 400                      # 16B-aligned padded stride for fp8 lhsT tiles
NPP = 200                      # padded NP stride inside E8

WS = 32.0               # fp8 weight scale
ES = 32.0               # normalized-prob fp8 scale
VS = 8.0                # v fp8 scale
OS = 4.0                # AV psum divided by OS at the po copy

# th_pack column offsets (all [128, x] chunk-major)
OFF_QKB = 0       # 12 (q bias pre-scaled by SCALE; LN-bias folded)
OFF_F1B = 12      # 24 (LN-bias folded)
OFF_G1 = 36       # 6: g1/(ES*VS/OS*WS)
OFF_G1PB = 42     # 6: g1*projb
OFF_G2 = 48       # 6: g2/WS
OFF_G2F2B = 54    # 6: g2*f2b
OFF_PLW = 60      # 72: (g,kchunk) -> plw[g, 2k + p//64]
OFF_PLB = 132     # 12
OFF_PWB = 144     # 12: ES*pwb
PCOLS = 156

# ca_pack offsets
CA_KB = 0     # 6
CA_G1 = 6     # 6: tg1/(ES*VS/OS*WS)
CA_G1PB = 12  # 6
CA_G2 = 18    # 6: tg2/WS
CA_G2F2B = 24 # 6
CA_QB = 30    # 12: (m,b) layout, q bias scaled
CACOLS = 42


def _f32(x):
    return np.ascontiguousarray(np.asarray(x, np.float32))


def _bf(x):
    return np.ascontiguousarray(np.asarray(x, np.float32)).astype(
        ml_dtypes.bfloat16)


def _fp8(x):
    return np.clip(np.asarray(x, np.float32), -240, 240).astype(
        ml_dtypes.float8_e4m3)


def _pmajor(w):
    """[C_in, X] -> [128, (C_in/128)*X] p-major (chunk-major free)."""
    cin, x = w.shape
    ch = cin // 128
    return w.reshape(ch, 128, x).transpose(1, 0, 2).reshape(128, ch * x)


def _pack_cols(vecs_768):
    cols = []
    for v in vecs_768:
        cols.append(np.asarray(v, np.float32).reshape(CH, 128).T)
    return np.concatenate(cols, axis=1)


def host_prep(inp):
    d = {}
    g = {k: np.asarray(v, np.float32) for k, v in inp.items()}

    # ---- patch embed ----
    d['patch_w8'] = _fp8(_pmajor(g['patch_w'].T * WS))   # [128, 6*768]
    posT = g['pos_embed'][0].T                           # [768, 196]
    posb = np.concatenate([posT, posT], axis=1) + g['patch_b'][:, None]
    d['posb'] = _f32(_pmajor(posb))                      # [128, 6*392]
    d['clsT'] = _f32(_pmajor(np.tile(g['cls_token'][0, 0][:, None], (1, BL))))

    # ---- talking-heads layers ----
    n1w, n1b = g['n1w'], g['n1b']
    n2w, n2b = g['n2w'], g['n2b']
    qkvw, qkvb = g['qkvw'], g['qkvb']
    projw, projb = g['projw'], g['projb']
    f1w, f1b, f2w, f2b = g['f1w'], g['f1b'], g['f2w'], g['f2b']
    g1, g2 = g['g1'], g['g2']
    plw, plb, pww, pwb = g['plw'], g['plb'], g['pww'], g['pwb']

    wqk_l, wv_l, proj_l, f1_l, f2_l, ids_l, pack_l, vb_l = ([] for _ in
                                                            range(8))
    for L in range(DEPTH):
        wqk = qkvw[L, :2 * C, :] * n1w[L][None, :]       # [1536, 768]
        bqk = qkvw[L, :2 * C, :] @ n1b[L] + qkvb[L, :2 * C]
        wqk[:C] *= SCALE
        bqk[:C] *= SCALE
        wqk_l.append(_pmajor(wqk.T * WS))                # [128, 6*1536]
        wv = qkvw[L, 2 * C:, :] * n1w[L][None, :]
        bv = qkvw[L, 2 * C:, :] @ n1b[L] + qkvb[L, 2 * C:]
        wv_l.append(_pmajor(wv.T * WS))                  # [128, 6*768]
        vb_l.append(np.tile((VS * bv)[None, :], (128, 1)))
        proj_l.append(_pmajor(projw[L].T * WS))
        wf1 = f1w[L] * n2w[L][None, :]
        bf1 = f1w[L] @ n2b[L] + f1b[L]
        f1_l.append(_pmajor(wf1.T * WS))                 # [128, 6*3072]
        f2_l.append(_pmajor(f2w[L].T * WS))              # [128, 24*768]

        ids = np.zeros((128, 72, 2, 128), np.float32)
        r = np.arange(128)
        for gh in range(HEADS):
            for jp in range(6):
                ids[r, gh * 6 + jp, 0, r] = VS * pww[L, gh, 2 * jp]
                ids[r, gh * 6 + jp, 1, r] = VS * pww[L, gh, 2 * jp + 1]
        ids_l.append(ids.reshape(128, 72 * 256))

        pl = np.zeros((128, 72), np.float32)
        for gh in range(HEADS):
            for k in range(CH):
                pl[:64, gh * 6 + k] = plw[L, gh, 2 * k]
                pl[64:, gh * 6 + k] = plw[L, gh, 2 * k + 1]
        pack = np.concatenate([
            bqk.reshape(12, 128).T,                      # 12
            bf1.reshape(FCH, 128).T,                     # 24
            _pack_cols([g1[L] / (ES * VS / OS * WS), g1[L] * projb[L],
                        g2[L] / WS, g2[L] * f2b[L]]),    # 24
            pl,                                          # 72
            np.tile(plb[L][None, :], (128, 1)),          # 12
            np.tile((ES * pwb[L])[None, :], (128, 1)),   # 12
        ], axis=1)
        pack_l.append(pack)

    d['wqk8'] = _fp8(np.stack(wqk_l))
    d['wv8'] = _fp8(np.stack(wv_l))
    d['proj8'] = _fp8(np.stack(proj_l))
    d['f1w8'] = _fp8(np.stack(f1_l))
    d['f2w8'] = _fp8(np.stack(f2_l))
    d['ids8'] = _fp8(np.stack(ids_l))
    d['th_pack'] = _f32(np.stack(pack_l))
    d['vb_bc'] = _f32(np.stack(vb_l))

    # ---- class-attention layers ----
    tn1w, tn1b = g['tn1w'], g['tn1b']
    tn2w, tn2b = g['tn2w'], g['tn2b']
    tw_l, tf1_l, tf2_l, cap_l, cvb_l, cf1b_l = ([] for _ in range(6))
    for L in range(D2):
        tq = g['tqw'][L] * tn1w[L][None, :]
        bq = (g['tqw'][L] @ tn1b[L] + g['tqb'][L]) * SCALE
        tk = g['tkw'][L] * tn1w[L][None, :]
        bk = g['tkw'][L] @ tn1b[L] + g['tkb'][L]
        tv = g['tvw'][L] * tn1w[L][None, :]
        bv = g['tvw'][L] @ tn1b[L] + g['tvb'][L]
        tw = np.concatenate([tq.T * SCALE, tk.T, tv.T, g['tprojw'][L].T],
                            axis=1)                      # [768, 3072]
        tw_l.append(_pmajor(tw * WS))
        cvb_l.append(np.tile((VS * bv)[None, :], (128, 1)))
        wtf1 = g['tf1w'][L] * tn2w[L][None, :]
        btf1 = g['tf1w'][L] @ tn2b[L] + g['tf1b'][L]
        tf1_l.append(_pmajor(wtf1.T * WS))
        tf2_l.append(_pmajor(g['tf2w'][L].T * WS))
        cf1b_l.append((WS * btf1)[None, :])              # [1, 3072]
        qb2 = np.repeat(bq.reshape(CH, 128).T, BL, axis=1)  # [128, 12] (m,b)
        cap_l.append(np.concatenate([
            _pack_cols([bk, g['tg1'][L] / (ES * VS / OS * WS),
                        g['tg1'][L] * g['tprojb'][L],
                        g['tg2'][L] / WS, g['tg2'][L] * g['tf2b'][L]]),
            qb2], axis=1))
    d['tw8'] = _fp8(np.stack(tw_l))
    d['tf18'] = _fp8(np.stack(tf1_l))
    d['tf28'] = _fp8(np.stack(tf2_l))
    d['ca_pack'] = _f32(np.stack(cap_l))
    d['ca_vb'] = _f32(np.stack(cvb_l))
    d['ca_f1b'] = _bf(np.stack(cf1b_l))                  # [2, 1, 3072]

    # ---- final (norm folded into head) ----
    hw = g['headw'] * g['normw'][None, :]
    hb = g['headw'] @ g['normb'] + g['headb']
    d['headwT'] = _bf(_pmajor(hw.T))                     # [128, 6*1000]
    d['headb2'] = _f32(np.tile(hb[None, :], (BL, 1)))
    d['cst_col'] = _f32(np.full((128, 1), 1.0 / C))
    d['cst_row'] = _f32(np.ones((1, 128)))
    return d


def host_pT(x_slice):
    """[2,3,224,224] -> fp8 [128, 6*392] patch matrix (p-major)."""
    p = np.asarray(x_slice, np.float32).reshape(BL, 3, GH, P_, GH, P_)
    p = p.transpose(0, 2, 4, 1, 3, 5).reshape(BL, NP, 3 * P_ * P_)
    return _fp8(_pmajor(p.transpose(2, 0, 1).reshape(3 * P_ * P_, BL * NP)))


INPUT_SPECS = [
    ('pT', [128, CH * TC], FP8), ('posb', [128, CH * TC], F32R),
    ('clsT', [128, CH * BL], F32R), ('patch_w8', [128, CH * C], FP8),
    ('wqk8', [DEPTH, 128, CH * 2 * C], FP8),
    ('wv8', [DEPTH, 128, CH * C], FP8),
    ('proj8', [DEPTH, 128, CH * C], FP8),
    ('f1w8', [DEPTH, 128, BIGW], FP8),
    ('f2w8', [DEPTH, 128, BIGW], FP8),
    ('ids8', [DEPTH, 128, 72 * 256], FP8),
    ('th_pack', [DEPTH, 128, PCOLS], F32),
    ('vb_bc', [DEPTH, 128, C], F32),
    ('tw8', [D2, 128, BIGW], FP8),
    ('tf18', [D2, 128, BIGW], FP8),
    ('tf28', [D2, 128, BIGW], FP8),
    ('ca_pack', [D2, 128, CACOLS], F32),
    ('ca_vb', [D2, 128, C], F32),
    ('ca_f1b', [D2, 1, FF], BF16),
    ('headwT', [128, CH * NCLS], BF16),
    ('headb2', [BL, NCLS], F32),
    ('cst_col', [128, 1], F32R),
    ('cst_row', [1, 128], F32R),
]


def build_program(depth=DEPTH, d2=D2, repeat=1):
    nc = bacc.Bacc("TRN2", target_bir_lowering=False, debug=False,
                   num_devices=NCORES)
    aps = {}
    for name, shape, dt in INPUT_SPECS:
        aps[name] = nc.dram_tensor(name, shape, dt, kind="ExternalInput").ap()
    out_ap = nc.dram_tensor("out", [BL, NCLS], F32, kind="ExternalOutput").ap()
    with tile.TileContext(nc) as tc:
        with ExitStack() as es, nc.allow_low_precision(reason="f32r LN stats"):
            emit_kernel(es, tc, out_ap, aps, depth, d2, repeat)
    nc.compile()
    return nc


def emit_kernel(es, tc, out_ap, aps, depth, d2, repeat=1):
    nc = tc.nc
    pers = es.enter_context(tc.tile_pool(name='pers', bufs=1))
    wpre = es.enter_context(tc.tile_pool(name='wpre', bufs=2))
    ap_ = es.enter_context(tc.tile_pool(name='act', bufs=1))
    sq_p = es.enter_context(tc.tile_pool(name='sq', bufs=1))
    tmp = es.enter_context(tc.tile_pool(name='tmp', bufs=2))
    tmp1 = es.enter_context(tc.tile_pool(name='tmp1', bufs=1))
    atp = es.enter_context(tc.tile_pool(name='atp', bufs=3))
    psA = es.enter_context(tc.tile_pool(name='psA', bufs=6, space="PSUM"))
    psB = es.enter_context(tc.tile_pool(name='psB', bufs=2, space="PSUM"))

    # ---- persistent tiles ----
    hT = pers.tile([128, CH, TC], F32R, tag='hT')
    xext = pers.tile([128, CH, TCA], F32R, tag='xext')
    u8 = pers.tile([128, CH, TCP], FP8, tag='u8')
    clsT = pers.tile([128, CH, BL], F32R, tag='clsT')
    big0 = pers.tile([128, BIGW], FP8, tag='big0')       # f1 / tw / tf2
    big1 = pers.tile([128, BIGW], FP8, tag='big1')       # f2 / tf1
    ids = pers.tile([128, 72, 2, 128], FP8, tag='ids')
    pjt = pers.tile([128, CH * C], FP8, tag='pjt')       # proj / patch_w
    hw = pers.tile([128, CH, NCLS], BF16, tag='hw')
    hb = pers.tile([BL, NCLS], F32, tag='hb')
    out_t = pers.tile([BL, NCLS], F32, tag='outt')
    ones_mean = pers.tile([128, 1], F32R, tag='onem')
    ones_row = pers.tile([1, 128], F32R, tag='oner')
    ones_z = pers.tile([128, 1], BF16, tag='onez')
    ones_b2 = pers.tile([1, BL], BF16, tag='oneb2')
    epsc = pers.tile([1, 1], F32, tag='epsc')
    nc.vector.memset(epsc[:], EPS)
    nc.sync.dma_start(out=ones_mean[:], in_=aps['cst_col'][:, :])
    nc.sync.dma_start(out=ones_row[:], in_=aps['cst_row'][:, :])
    nc.vector.memset(ones_z[:], 1.0 / ES)
    nc.vector.memset(ones_b2[:], 1.0)

    f1v = big0[:].rearrange("p (c x) -> p c x", c=CH)     # [128, 6, 3072]
    f2v = big1[:].rearrange("p (c x) -> p c x", c=FCH)    # [128, 24, 768]
    twv = big0[:].rearrange("p (c x) -> p c x", c=CH)     # q|k|v|proj
    tf1v = big1[:].rearrange("p (c x) -> p c x", c=CH)
    tf2v = big0[:].rearrange("p (c x) -> p c x", c=FCH)
    pjv = pjt[:].rearrange("p (c x) -> p c x", c=CH)      # [128, 6, 768]

    def dma(dst, src):
        nc.sync.dma_start(out=dst, in_=src)

    # ---------- layernorm helpers (transposed layout) ----------
    def ln_stats(x, cols, tagn):
        """x f32r tile view [128, CH, cols] -> psum (Rb, Mb) [128, cols]."""
        s1 = psA.tile([1, cols], F32, tag='pa')
        s2 = psA.tile([1, cols], F32, tag='pa')
        for c in range(CH):
            nc.tensor.matmul(s1[:], ones_mean[:], x[:, c, :],
                             start=(c == 0), stop=(c == CH - 1))
        for c in range(CH):
            sq = sq_p.tile([128, cols], F32R, tag='lnsq')
            nc.gpsimd.tensor_tensor(sq[:], x[:, c, :], x[:, c, :], ALU.mult)
            nc.tensor.matmul(s2[:], ones_mean[:], sq[:],
                             start=(c == 0), stop=(c == CH - 1))
        rows = tmp1.tile([1, 3, cols], F32R, tag='lnr' + tagn,
                         name='lnr' + tagn)
        m2, r, mr = rows[:, 0, :], rows[:, 1, :], rows[:, 2, :]
        nc.scalar.square(m2, s1[:])                       # mu^2
        nc.vector.tensor_sub(m2, s2[:], m2)               # var
        nc.scalar.activation(m2, m2, AF.Ln, bias=epsc[:])  # ln(var+eps)
        nc.scalar.activation(r, m2, AF.Exp, scale=-0.5)   # rstd
        nc.vector.scalar_tensor_tensor(mr, s1[:], -1.0, r, ALU.mult, ALU.mult)
        Rb = psB.tile([128, cols], F32, tag='pb')
        Mb = psB.tile([128, cols], F32, tag='pb')
        nc.tensor.matmul(Rb[:], ones_row[:], r, start=True, stop=True)
        nc.tensor.matmul(Mb[:], ones_row[:], mr, start=True, stop=True)
        return Rb, Mb

    def xhat_to(dst, x, cols, tagn):
        """LN x-hat of x into dst tile [128, CH, cols] (any dtype)."""
        Rb, Mb = ln_stats(x, cols, tagn)
        for c in range(CH):
            t1 = tmp.tile([128, cols], F32, tag='xh1')
            nc.vector.tensor_mul(t1[:], x[:, c, :], Rb[:])
            nc.vector.tensor_add(dst[:, c, :cols], t1[:], Mb[:])

    for _rep in range(repeat):
        # ---------- patch embed ----------
        pt = ap_.tile([128, CH, TCP], FP8, tag='y')
        posb = xext[:, :, :TC]
        dma(pt[:, :, :TC], aps['pT'][:, :].rearrange("p (c x) -> p c x", c=CH))
        dma(pjt[:], aps['patch_w8'][:, :])
        dma(posb, aps['posb'][:, :].rearrange("p (c x) -> p c x", c=CH))
        for m in range(CH):
            pm = psA.tile([128, TC], F32, tag='pa')
            for kp in range(KP):
                nc.tensor.matmul(
                    pm[:], pjv[:, 2 * kp:2 * kp + 2, m * 128:(m + 1) * 128],
                    pt[:, 2 * kp:2 * kp + 2, :TC],
                    start=(kp == 0), stop=(kp == KP - 1), perf_mode=DR)
            nc.vector.scalar_tensor_tensor(hT[:, m, :], pm[:], 1.0 / WS,
                                           posb[:, m, :TC], ALU.mult, ALU.add)

        # ---------- talking-heads blocks ----------
        for L in range(depth):
            pk = wpre.tile([128, PCOLS], F32, tag='pk')
            dma(pk[:], aps['th_pack'][L])
            vbb = wpre.tile([128, C], F32, tag='vbb')
            dma(vbb[:], aps['vb_bc'][L])
            wqk = wpre.tile([128, CH, 2 * C], FP8, tag='wqk')
            dma(wqk[:], aps['wqk8'][L].rearrange("p (c x) -> p c x", c=CH))
            wv = wpre.tile([128, CH, C], FP8, tag='wv')
            dma(wv[:], aps['wv8'][L].rearrange("p (c x) -> p c x", c=CH))
            dma(pjt[:], aps['proj8'][L])
            dma(ids[:].rearrange("p g t x -> p (g t x)"), aps['ids8'][L])
            dma(big0[:], aps['f1w8'][L])
            dma(big1[:], aps['f2w8'][L])

            y = ap_.tile([128, CH, TCP], FP8, tag='y')
            xhat_to(y, hT, TC, 'a')

            # qk projection -> qk [128, 12, 392] bf16 (q pre-scaled)
            qk = ap_.tile([128, 2 * CH, TC], BF16, tag='qk')
            for m in range(2 * CH):
                pm = psA.tile([128, TC], F32, tag='pa')
                for kp in range(KP):
                    nc.tensor.matmul(
                        pm[:], wqk[:, 2 * kp:2 * kp + 2,
                                   m * 128:(m + 1) * 128],
                        y[:, 2 * kp:2 * kp + 2, :TC],
                        start=(kp == 0), stop=(kp == KP - 1), perf_mode=DR)
                nc.vector.tensor_scalar(
                    qk[:, m, :], pm[:], 1.0 / WS,
                    pk[:, OFF_QKB + m:OFF_QKB + m + 1], ALU.mult, ALU.add)

            # v natural [tok, C] fp8 (*VS); slot (b*2+ci)
            vnat = ap_.tile([128, 2 * BL, C], FP8, tag='vnat')
            for b in range(BL):
                for ci, (noff, nsz) in enumerate(NCH):
                    for half in range(2):
                        pv = psA.tile([128, 384], F32, tag='pa')
                        for kp in range(KP):
                            nc.tensor.matmul(
                                pv[:nsz, :],
                                y[:, 2 * kp:2 * kp + 2,
                                  b * NP + noff:b * NP + noff + nsz],
                                wv[:, 2 * kp:2 * kp + 2,
                                   half * 384:(half + 1) * 384],
                                start=(kp == 0), stop=(kp == KP - 1),
                                perf_mode=DR)
                        nc.vector.scalar_tensor_tensor(
                            vnat[:nsz, b * 2 + ci,
                                 half * 384:(half + 1) * 384],
                            pv[:nsz, :], VS / WS,
                            vbb[:nsz, half * 384:(half + 1) * 384],
                            ALU.mult, ALU.add)

            # premixed scores (bf16) -> exp (fp8) -> Z -> normalize in-place
            E8 = [ap_.tile([128, HEADS, BL, NPP], FP8, tag=f'E8{ci}',
                           name=f'E8{ci}_{L}') for ci in range(2)]
            Z = [tmp.tile([128, HEADS, BL], F32, tag=f'Z{ci}',
                          name=f'Z{ci}_{L}') for ci in range(2)]
            rZ = [tmp.tile([128, HEADS, BL], F32, tag=f'rZ{ci}',
                           name=f'rZ{ci}_{L}') for ci in range(2)]
            for gh in range(HEADS):
                sq = sq_p.tile([128, CH, TC], BF16, tag='sq')
                for k in range(CH):
                    nc.vector.tensor_scalar(
                        sq[:, k, :], qk[:, k, :],
                        pk[:, OFF_PLW + gh * 6 + k:OFF_PLW + gh * 6 + k + 1],
                        None, ALU.mult)
                for ci, (noff, nsz) in enumerate(NCH):
                    pm = psA.tile([128, BL, NP], F32, tag='pa')
                    for b in range(BL):
                        for k in range(CH):
                            nc.tensor.matmul(
                                pm[:nsz, b, :],
                                sq[:, k, b * NP + noff:b * NP + noff + nsz],
                                qk[:, CH + k, b * NP:(b + 1) * NP],
                                start=(k == 0), stop=(k == CH - 1))
                    nc.scalar.activation(
                        E8[ci][:nsz, gh, :, :NP], pm[:nsz, :, :], AF.Exp,
                        bias=pk[:nsz, OFF_PLB + gh:OFF_PLB + gh + 1])
                    nc.vector.tensor_reduce(
                        Z[ci][:nsz, gh, :], E8[ci][:nsz, gh, :, :NP], AXX,
                        ALU.add)
            for ci, (noff, nsz) in enumerate(NCH):
                nc.vector.reciprocal(rZ[ci][:nsz, :, :], Z[ci][:nsz, :, :])
            for gh in range(HEADS):
                for b in range(BL):
                    for ci, (noff, nsz) in enumerate(NCH):
                        nc.gpsimd.tensor_scalar(
                            E8[ci][:nsz, gh, b, :NP], E8[ci][:nsz, gh, b, :NP],
                            rZ[ci][:nsz, gh, b:b + 1], ES, ALU.mult, ALU.mult)

            # post-softmax mix (fp8 DR scaled-identity j-pairs; transposes
            # to [key, query]) + AV (fp8 DR over key chunk-pairs)
            oT = ap_.tile([128, CH, TC], FP8, tag='oT')
            for gh in range(HEADS):
                at = atp.tile([128, 2, BL, NP], FP8, tag='at',
                              name=f'at{gh}_{L}')
                for mi, (moff, msz) in enumerate(NCH):
                    pp = psA.tile([128, BL, NP], F32, tag='pa')
                    for b in range(BL):
                        for ci, (noff, nsz) in enumerate(NCH):
                            for jp in range(6):
                                nc.tensor.matmul(
                                    pp[:msz, b, noff:noff + nsz],
                                    E8[ci][:nsz, 2 * jp:2 * jp + 2, b,
                                           moff:moff + msz],
                                    ids[:nsz, gh * 6 + jp, :, :nsz],
                                    start=(jp == 0 and ci == 0),
                                    stop=(jp == 5 and ci == 1),
                                    perf_mode=DR)
                    nc.scalar.activation(
                        at[:msz, mi, :, :], pp[:msz, :, :], AF.Identity,
                        bias=pk[:msz, OFF_PWB + gh:OFF_PWB + gh + 1])
                po = psA.tile([64, BL, NP], F32, tag='pa')
                for b in range(BL):
                    for ci, (noff, nsz) in enumerate(NCH):
                        nc.tensor.matmul(
                            po[:, b, :],
                            vnat[:nsz, b * 2 + ci, gh * 64:(gh + 1) * 64],
                            at[:nsz, ci, b, :], start=(ci == 0),
                            stop=(ci == 1))
                nc.vector.tensor_scalar(
                    oT[(gh % 2) * 64:(gh % 2) * 64 + 64, gh // 2, :],
                    po[:, :, :], 1.0 / OS, None, ALU.mult)

            # attn out projection + residual
            for m in range(CH):
                pm = psA.tile([128, TC], F32, tag='pa')
                for kp in range(KP):
                    nc.tensor.matmul(
                        pm[:], pjv[:, 2 * kp:2 * kp + 2,
                                    m * 128:(m + 1) * 128],
                        oT[:, 2 * kp:2 * kp + 2, :],
                        start=(kp == 0), stop=(kp == KP - 1), perf_mode=DR)
                nc.vector.affine_then_add(
                    hT[:, m, :], pm[:], hT[:, m, :],
                    scale=pk[:, OFF_G1 + m:OFF_G1 + m + 1],
                    bias=pk[:, OFF_G1PB + m:OFF_G1PB + m + 1])

            # MLP
            y2 = ap_.tile([128, CH, TCP], FP8, tag='y')
            xhat_to(y2, hT, TC, 'a')
            gl = ap_.tile([128, FCH, TC], FP8, tag='gl')
            for kf in range(FCH):
                pm = psA.tile([128, TC], F32, tag='pa')
                for kp in range(KP):
                    nc.tensor.matmul(
                        pm[:], f1v[:, 2 * kp:2 * kp + 2,
                                   kf * 128:(kf + 1) * 128],
                        y2[:, 2 * kp:2 * kp + 2, :TC],
                        start=(kp == 0), stop=(kp == KP - 1), perf_mode=DR)
                nc.scalar.activation(gl[:, kf, :], pm[:], AF.Gelu,
                                     bias=pk[:, OFF_F1B + kf:OFF_F1B + kf + 1],
                                     scale=1.0 / WS)
            for m in range(CH):
                pm = psA.tile([128, TC], F32, tag='pa')
                for kp in range(FCH // 2):
                    nc.tensor.matmul(
                        pm[:], f2v[:, 2 * kp:2 * kp + 2,
                                   m * 128:(m + 1) * 128],
                        gl[:, 2 * kp:2 * kp + 2, :],
                        start=(kp == 0), stop=(kp == FCH // 2 - 1),
                        perf_mode=DR)
                nc.vector.affine_then_add(
                    hT[:, m, :], pm[:], hT[:, m, :],
                    scale=pk[:, OFF_G2 + m:OFF_G2 + m + 1],
                    bias=pk[:, OFF_G2F2B + m:OFF_G2F2B + m + 1])

        # ---------- x-hat of frozen h -> xext cols; cast to u8 ----------
        Rb, Mb = ln_stats(hT, TC, 'a')
        for c in range(CH):
            t1 = tmp.tile([128, TC], F32, tag='xh1')
            nc.vector.tensor_mul(t1[:], hT[:, c, :], Rb[:])
            for b in range(BL):
                nc.vector.tensor_add(
                    xext[:, c, b * TN + 1:(b + 1) * TN],
                    t1[:, b * NP:(b + 1) * NP], Mb[:, b * NP:(b + 1) * NP])
        dma(clsT[:], aps['clsT'][:, :].rearrange("p (c x) -> p c x", c=CH))
        for c in range(CH):
            for b in range(BL):
                nc.vector.tensor_copy(u8[:, c, b * TN + 1:(b + 1) * TN],
                                      xext[:, c, b * TN + 1:(b + 1) * TN])

        def cls_xhat(dst, tagn):
            Rb, Mb = ln_stats(clsT, BL, tagn)
            for c in range(CH):
                t1 = tmp.tile([128, BL], F32, tag='ct1')
                nc.vector.tensor_mul(t1[:], clsT[:, c, :], Rb[:])
                nc.vector.tensor_add(dst[:, c, :], t1[:], Mb[:])

        # ---------- class-attention blocks ----------
        u4 = u8[:, :, :TCA].rearrange("p c (b t) -> p c b t", b=BL)
        for l in range(d2):
            cap = wpre.tile([128, CACOLS], F32, tag='cap')
            dma(cap[:], aps['ca_pack'][l])
            cvb = wpre.tile([128, C], F32, tag='vbb')
            dma(cvb[:], aps['ca_vb'][l])
            cf1b = wpre.tile([1, FF], BF16, tag='cf1b')
            dma(cf1b[:], aps['ca_f1b'][l][0])
            dma(big0[:], aps['tw8'][l])
            dma(big1[:], aps['tf18'][l])

            # u8 cls cols <- cls x-hat
            uc = tmp.tile([128, CH, BL], F32, tag='uc')
            cls_xhat(uc, 'c')
            for c in range(CH):
                nc.vector.tensor_copy(u4[:, c, :, 0], uc[:, c, :])

            # k-projection -> kv bf16 [128, 6, TCA]
            kv = ap_.tile([128, CH, TCA], BF16, tag='kv')
            for m in range(CH):
                pm = psA.tile([128, TCA], F32, tag='pa')
                for kp in range(KP):
                    nc.tensor.matmul(
                        pm[:], twv[:, 2 * kp:2 * kp + 2,
                                   C + m * 128:C + (m + 1) * 128],
                        u8[:, 2 * kp:2 * kp + 2, :TCA],
                        start=(kp == 0), stop=(kp == KP - 1), perf_mode=DR)
                nc.vector.tensor_scalar(
                    kv[:, m, :], pm[:], 1.0 / WS,
                    cap[:, CA_KB + m:CA_KB + m + 1], ALU.mult, ALU.add)
            # v natural fp8 (*VS)
            vnat = ap_.tile([128, 2 * BL, C], FP8, tag='vnat')
            for b in range(BL):
                for ci, (toff, tsz) in enumerate(TCH):
                    for half in range(2):
                        pv = psA.tile([128, 384], F32, tag='pa')
                        for kp in range(KP):
                            nc.tensor.matmul(
                                pv[:tsz, :],
                                u8[:, 2 * kp:2 * kp + 2,
                                   b * TN + toff:b * TN + toff + tsz],
                                twv[:, 2 * kp:2 * kp + 2,
                                    2 * C + half * 384:2 * C + (half + 1) * 384],
                                start=(kp == 0), stop=(kp == KP - 1),
                                perf_mode=DR)
                        nc.vector.scalar_tensor_tensor(
                            vnat[:tsz, b * 2 + ci,
                                 half * 384:(half + 1) * 384],
                            pv[:tsz, :], VS / WS,
                            cvb[:tsz, half * 384:(half + 1) * 384],
                            ALU.mult, ALU.add)

            # q -> qT bf16 [128, 6, 2] -> scattered into qz [128, 6, 12]
            pq = psA.tile([128, CH, BL], F32, tag='pa')
            for m in range(CH):
                for kp in range(KP):
                    nc.tensor.matmul(
                        pq[:, m, :],
                        twv[:, 2 * kp:2 * kp + 2, m * 128:(m + 1) * 128],
                        u4[:, 2 * kp:2 * kp + 2, :, 0],
                        start=(kp == 0), stop=(kp == KP - 1), perf_mode=DR)
            qT = tmp.tile([128, CH, BL], BF16, tag='qT')
            nc.vector.scalar_tensor_tensor(
                qT[:, :, :], pq[:, :, :], 1.0 / WS,
                cap[:, CA_QB:CA_QB + CH * BL].rearrange(
                    "p (c b) -> p c b", b=BL), ALU.mult, ALU.add)
            oTc = ap_.tile([128, CH, BL], FP8, tag='oTc')
            qz = tmp.tile([128, CH, HEADS], BF16, tag='qz')
            for b in range(BL):
                nc.vector.memset(qz[:], 0.0)
                qzf = qz[:].rearrange("p c h -> p (c h)")
                for r in range(2):
                    nc.vector.tensor_copy(
                        qzf[r * 64:(r + 1) * 64, r::HEADS + 2],
                        qT[r * 64:(r + 1) * 64, :, b])
                # scoresT [tok, 12] = kv^T qz ; exp ; Z ; normalize
                ET = atp.tile([128, 2, HEADS], BF16, tag='ET',
                              name=f'ET{l}_{b}')
                En8 = atp.tile([128, 2, HEADS], FP8, tag='En8',
                               name=f'En8{l}_{b}')
                pz = psA.tile([1, 2, HEADS], F32, tag='pa')
                for ci, (toff, tsz) in enumerate(TCH):
                    ps = psA.tile([128, HEADS], F32, tag='pa')
                    for k in range(CH):
                        nc.tensor.matmul(
                            ps[:tsz, :],
                            kv[:, k, b * TN + toff:b * TN + toff + tsz],
                            qz[:, k, :], start=(k == 0), stop=(k == CH - 1))
                    nc.scalar.activation(ET[:tsz, ci, :], ps[:tsz, :], AF.Exp)
                    nc.tensor.matmul(pz[:, ci, :], ones_z[:tsz],
                                     ET[:tsz, ci, :], start=True, stop=True)
                rz = tmp.tile([1, 2, HEADS], F32R, tag='rz')
                nc.vector.reciprocal(rz[:], pz[:])       # ES/Z
                rzb = psB.tile([128, 2 * HEADS], F32, tag='pb')
                nc.tensor.matmul(rzb[:], ones_row[:],
                                 rz[:].rearrange("p a h -> p (a h)"),
                                 start=True, stop=True)
                rzb3 = rzb[:].rearrange("p (a h) -> p a h", a=2)
                for ci, (toff, tsz) in enumerate(TCH):
                    nc.vector.tensor_mul(En8[:tsz, ci, :], ET[:tsz, ci, :],
                                         rzb3[:tsz, ci, :])
                # AV: po [64, 12] (fp8 DR over chunk pair)
                po = psA.tile([64, HEADS], F32, tag='pa')
                for h in range(HEADS):
                    for ci, (toff, tsz) in enumerate(TCH):
                        nc.tensor.matmul(
                            po[:, h:h + 1],
                            vnat[:tsz, b * 2 + ci, h * 64:(h + 1) * 64],
                            En8[:tsz, ci, h:h + 1], start=(ci == 0),
                            stop=(ci == 1))
                for r in range(2):
                    nc.scalar.activation(
                        oTc[r * 64:(r + 1) * 64, :, b],
                        po[:, r::2], AF.Identity, scale=VS / OS)

            # proj + residual into clsT
            pm = psA.tile([128, CH, BL], F32, tag='pa')
            for m in range(CH):
                for kp in range(KP):
                    nc.tensor.matmul(
                        pm[:, m, :],
                        twv[:, 2 * kp:2 * kp + 2,
                            3 * C + m * 128:3 * C + (m + 1) * 128],
                        oTc[:, 2 * kp:2 * kp + 2, :],
                        start=(kp == 0), stop=(kp == KP - 1), perf_mode=DR)
            for m in range(CH):
                nc.vector.affine_then_add(
                    clsT[:, m, :], pm[:, m, :], clsT[:, m, :],
                    scale=cap[:, CA_G1 + m:CA_G1 + m + 1],
                    bias=cap[:, CA_G1PB + m:CA_G1PB + m + 1])

            dma(big0[:], aps['tf28'][l])

            # cls MLP
            u2 = tmp.tile([128, CH, BL], FP8, tag='u2')
            cls_xhat(u2, 'd')
            pgl = psA.tile([128, FCH, BL], F32, tag='pa')
            for kf in range(FCH):
                nc.tensor.matmul(pgl[:, kf, :],
                                 cf1b[:, kf * 128:(kf + 1) * 128],
                                 ones_b2[:], start=True, stop=False,
                                 skip_group_check=True)
                for kp in range(KP):
                    nc.tensor.matmul(
                        pgl[:, kf, :],
                        tf1v[:, 2 * kp:2 * kp + 2, kf * 128:(kf + 1) * 128],
                        u2[:, 2 * kp:2 * kp + 2, :],
                        start=False, stop=(kp == KP - 1), perf_mode=DR,
                        skip_group_check=True)
            gl2 = tmp.tile([128, FCH, BL], FP8, tag='gl2')
            nc.scalar.activation(gl2[:, :, :], pgl[:, :, :], AF.Gelu,
                                 scale=1.0 / WS)
            pm2 = psA.tile([128, CH, BL], F32, tag='pa')
            for m in range(CH):
                for kp in range(FCH // 2):
                    nc.tensor.matmul(
                        pm2[:, m, :],
                        tf2v[:, 2 * kp:2 * kp + 2, m * 128:(m + 1) * 128],
                        gl2[:, 2 * kp:2 * kp + 2, :],
                        start=(kp == 0), stop=(kp == FCH // 2 - 1),
                        perf_mode=DR)
            for m in range(CH):
                nc.vector.affine_then_add(
                    clsT[:, m, :], pm2[:, m, :], clsT[:, m, :],
                    scale=cap[:, CA_G2 + m:CA_G2 + m + 1],
                    bias=cap[:, CA_G2F2B + m:CA_G2F2B + m + 1])

        # ---------- final head (norm folded) ----------
        zf = tmp.tile([128, CH, BL], BF16, tag='zf')
        cls_xhat(zf, 'e')
        dma(hw[:], aps['headwT'][:, :].rearrange("p (c x) -> p c x", c=CH))
        dma(hb[:], aps['headb2'])
        for nh in range(2):
            phd = psA.tile([BL, 500], F32, tag='pa')
            for k in range(CH):
                nc.tensor.matmul(phd[:], zf[:, k, :],
                                 hw[:, k, nh * 500:(nh + 1) * 500],
                                 start=(k == 0), stop=(k == CH - 1))
            nc.vector.tensor_add(out_t[:, nh * 500:(nh + 1) * 500], phd[:],
                                 hb[:, nh * 500:(nh + 1) * 500])
        dma(out_ap[:, :], out_t[:])


_NC_CACHE = {}


def kernel(**inputs):
    if 'full' not in _NC_CACHE:
        _NC_CACHE['full'] = build_program()
    nc = _NC_CACHE['full']
    shared = host_prep(inputs)
    x = np.asarray(inputs['x'], np.float32)
    in_maps = []
    for c in range(NCORES):
        m = dict(shared)
        m['pT'] = host_pT(x[c * BL:(c + 1) * BL])
        in_maps.append(m)
    res = run_bass_kernel_spmd(nc, in_maps, list(range(NCORES)))
    return np.concatenate([res.results[c]['out'] for c in range(NCORES)],
                          axis=0).astype(np.float32)


if __name__ == '__main__':
    import time
    t0 = time.time()
    build_program()
    print("traced+compiled ok in", time.time() - t0, "s")
